# revision 32
# baseline (speedup 1.0000x reference)
"""Distributed single-head attention for TRN2 (8 NeuronCores).

Reference computation (per batch b):
    q = x @ Wq; k = x @ Wk; v = x @ Wv          (x: [S, E])
    s = (q @ k.T) / sqrt(DK) - 1e15 * mask
    out = softmax(s, axis=-1) @ v               ([S, DV])

Sharding: 8 cores = 4 batches x 2 sequence halves. Each core computes
attention for 1024 queries of one batch; K/V are recomputed per core from
the full sequence (cheap vs. the attention matmuls, avoids collectives).

Per-core layout choices (host prepares these in kernel()):
  - xt  [E, S]  bf16: x_b^T with the sequence permuted so this core's
                query half occupies columns [0, 1024). K/V are computed
                over the permuted order, which is harmless because
                softmax-attention is permutation invariant over keys.
  - wq  [E, DK] bf16: Wq pre-scaled by 1/sqrt(DK).
  - mt  [S, SQ] bf16: mask[b, q0:q0+SQ, :] transposed to [key, query]
                with keys permuted identically to xt's columns.
  - out [P, SQ] f32: output queries grouped by 128: out[p, c*128+d] =
                attention_out[q0 + c*128 + p, d]; host re-interleaves.

On-core dataflow (all matmul contractions on the 128-partition dim):
  QT[d,q], KT[d,k], VT[d,k] projections -> DMA-transpose VT -> V[k,d]
  per key-tile t: ST[k128,q] = KT_t^T QT (PE) -> P = exp(ST) bf16 (ACT)
  -> zero masked entries (DVE copy_predicated) -> rowsum via ones-matmul
  and OT[dv,q] += V_t^T P (PE, PSUM accumulate). Epilogue transposes the
  rowsum into partition-major [128, 8] so the reciprocal runs on 8
  elements per lane (a [1, 1024] reciprocal costs 6.5us), then
  transposes OT chunks to [q, dv] and scales by 1/rowsum as a
  per-partition scalar.
"""

import math
from contextlib import ExitStack

import ml_dtypes
import numpy as np

import concourse.bass as bass
import concourse.bass_utils as _bass_utils
import concourse.tile as tile
from concourse import bacc, masks, mybir
from concourse.bass_utils import run_bass_kernel_spmd

# Note: tried forcing walrus --enable-ldw-opt=true to dedup the per-matmul
# LDWEIGHTS (~107ns each); walrus rejects it ("InstLdweights is not compatible
# with LDW optimization") because bacc emits standalone Ldweights when moving
# matmul waits. Keeping the default.
del _bass_utils

B, S, E, DK, DV = 4, 2048, 1024, 128, 128
SQ = S // 2  # queries per core
P = 128  # SBUF partitions
EC = E // P  # contraction chunks for projections
KTILES = S // P  # key tiles
QC = SQ // P  # query chunks (epilogue)

f32 = mybir.dt.float32
bf16 = mybir.dt.bfloat16

# test.py pokes these to get profiling info
TRACE = False
LAST_RESULT = None


def build():
    nc = bacc.Bacc()
    xt = nc.declare_dram_parameter("xt", [E, S], bf16, isOutput=False)
    # weights arrive host-packed as [p, c*DK+d] = W[c*128+p, d] so the load
    # is one fully-contiguous DMA (2KB/partition descriptors)
    wq = nc.declare_dram_parameter("wq", [P, EC * DK], bf16, isOutput=False)
    wk = nc.declare_dram_parameter("wk", [P, EC * DK], bf16, isOutput=False)
    wv = nc.declare_dram_parameter("wv", [P, EC * DV], bf16, isOutput=False)
    mt = nc.declare_dram_parameter("mt", [S, SQ], bf16, isOutput=False)
    out = nc.declare_dram_parameter("out", [P, SQ], f32, isOutput=True)

    with ExitStack() as ctx:
        tc = ctx.enter_context(tile.TileContext(nc))
        const_pool = ctx.enter_context(tc.tile_pool(name="const", bufs=1))
        in_pool = ctx.enter_context(tc.tile_pool(name="inputs", bufs=1))
        proj_sb = ctx.enter_context(tc.tile_pool(name="proj", bufs=1))
        p_pool = ctx.enter_context(tc.tile_pool(name="p", bufs=4))
        stat = ctx.enter_context(tc.tile_pool(name="stat", bufs=1))
        proj_ctx = ctx.enter_context(ExitStack())
        proj_ps = proj_ctx.enter_context(
            tc.tile_pool(name="proj_ps", bufs=8, space="PSUM")
        )

        ones_col = const_pool.tile([P, 1], bf16)
        nc.gpsimd.memset(ones_col[:], 1.0)
        zeros_p = const_pool.tile([P, SQ], bf16)
        nc.gpsimd.memset(zeros_p[:], 0.0)
        ident = const_pool.tile([P, P], f32)
        masks.make_identity(nc, ident[:])

        # --- weights + x chunks. One strided DMA per weight keeps the sync
        # sequencer's ~330ns/dma_start issue serialization off the critical
        # path; issue order wq, x0, wk, wv, x1.. matches first-use order.
        w_sb = {}
        for name, w in (("wq", wq), ("wk", wk), ("wv", wv)):
            wt = in_pool.tile([P, EC * DK], bf16, tag=name)
            w_sb[name] = wt

        x_sb = []
        for c in range(EC):
            xc = in_pool.tile([P, S], bf16, tag=f"x{c}")
            x_sb.append(xc)

        nc.sync.dma_start(w_sb["wq"][:], wq[:, :])
        nc.sync.dma_start(x_sb[0][:], xt[0:P, :])
        nc.sync.dma_start(w_sb["wk"][:], wk[:, :])
        nc.sync.dma_start(w_sb["wv"][:], wv[:, :])
        last_x_dma = None
        for c in range(1, EC):
            last_x_dma = nc.sync.dma_start(x_sb[c][:], xt[c * P : (c + 1) * P, :])

        # --- projections: QT [d, q], KT [d, k], VT [d, k] (bf16 in SBUF) ---
        # Contraction chunk c is the outer loop so the first matmul only
        # needs x chunk 0, not all eight.
        qt_sb = proj_sb.tile([P, SQ], bf16)
        kt_sb = proj_sb.tile([P, S], bf16)
        vt_sb = proj_sb.tile([P, S], bf16)

        # QT and KT interleaved per x-chunk: ~1.9us of matmuls per chunk
        # keeps PE just behind the ~1.4us/chunk DMA arrival rate. VT follows
        # (x already resident by then).
        def alloc_ps(n):
            pss = []
            for j in range(n):
                ps = proj_ps.tile([P, 512], f32, tag="pps")
                pss.append(ps)
            return pss

        def proj_mm(pss, w_tile, c, nj):
            for j in range(nj):
                nc.tensor.matmul(
                    pss[j][:],
                    w_tile[:, c * DK : (c + 1) * DK],
                    x_sb[c][:, j * 512 : (j + 1) * 512],
                    start=(c == 0),
                    stop=(c == EC - 1),
                )

        def proj_copy(dst, pss):
            for j, ps in enumerate(pss):
                nc.vector.tensor_copy(dst[:, j * 512 : (j + 1) * 512], ps[:])

        qt_ps = alloc_ps(2)
        kt_ps = alloc_ps(4)
        for c in range(EC):
            proj_mm(qt_ps, w_sb["wq"], c, 2)
            proj_mm(kt_ps, w_sb["wk"], c, 4)
        proj_copy(qt_sb, qt_ps)
        proj_copy(kt_sb, kt_ps)
        vt_ps = alloc_ps(4)
        for c in range(EC):
            proj_mm(vt_ps, w_sb["wv"], c, 4)
        proj_copy(vt_sb, vt_ps)
        proj_ctx.close()  # free projection PSUM banks for the attention loop

        # --- V natural layout [k, dv] via DMA transpose on the sync stream
        # (sync has nothing left to issue afterward except output stores, so
        # blocking on vt_sb readiness is harmless) ---
        v_sb = proj_sb.tile([P, S], bf16)  # tile t at columns [t*DV, (t+1)*DV)
        for t in range(KTILES):
            nc.sync.dma_start_transpose(
                v_sb[:, t * DV : (t + 1) * DV], vt_sb[:, t * P : (t + 1) * P]
            )

        # --- mask loads on the GPSIMD SWDGE stream (third parallel issuer);
        # needed only from the attention loop onward ---
        m_sb = []
        for t in range(KTILES):
            mtile = in_pool.tile([P, SQ], bf16, tag=f"m{t}")
            m_sb.append(mtile)
            m_dma = nc.gpsimd.dma_start(mtile[:], mt[t * P : (t + 1) * P, :])
            # hold mask traffic until x is fully resident — both streams
            # share the ~358GB/s HBM pipe and the projections are gated on x
            tile.add_dep_helper(
                last_x_dma.ins, m_dma.ins, reason="mask DMA after x loads"
            )

        st_ps = ctx.enter_context(tc.tile_pool(name="st_ps", bufs=2, space="PSUM"))
        ot_ps = ctx.enter_context(tc.tile_pool(name="ot_ps", bufs=1, space="PSUM"))
        rs_pool = ctx.enter_context(tc.tile_pool(name="rs_ps", bufs=1, space="PSUM"))

        # --- attention over key tiles ---
        ot = ot_ps.tile([P, SQ], f32)  # OT [dv, q] accumulator
        rs = rs_pool.tile([1, SQ], f32)  # rowsum of masked exp(scores)
        for t in range(KTILES):
            st = st_ps.tile([P, SQ], f32, tag="st")  # [k128, q]
            for j in range(2):
                nc.tensor.matmul(
                    st[:, j * 512 : (j + 1) * 512],
                    kt_sb[:, t * P : (t + 1) * P],
                    qt_sb[:, j * 512 : (j + 1) * 512],
                    start=True,
                    stop=True,
                )
            p = p_pool.tile([P, SQ], bf16, tag="p")
            nc.scalar.activation(p[:], st[:], mybir.ActivationFunctionType.Exp)
            # zero the masked entries: exp(s - 1e15*m) == exp(s) * (1 - m)
            nc.vector.copy_predicated(
                p[:], m_sb[t][:].bitcast(mybir.dt.uint16), zeros_p[:]
            )
            for j in range(2):
                nc.tensor.matmul(
                    rs[:, j * 512 : (j + 1) * 512],
                    ones_col[:],
                    p[:, j * 512 : (j + 1) * 512],
                    start=(t == 0),
                    stop=(t == KTILES - 1),
                )
                nc.tensor.matmul(
                    ot[:, j * 512 : (j + 1) * 512],
                    v_sb[:, t * DV : (t + 1) * DV],
                    p[:, j * 512 : (j + 1) * 512],
                    start=(t == 0),
                    stop=(t == KTILES - 1),
                )

        # --- epilogue: normalize in [q, dv] layout ---
        # rowsum [1, SQ] -> SBUF -> PE-transpose to [128, QC] so reciprocal
        # runs on QC elements per lane instead of SQ on one lane.
        rs_sb = stat.tile([1, SQ], f32)
        nc.scalar.copy(rs_sb[:], rs[:])
        rsT = st_ps.tile([P, QC], f32, tag="st")
        for c in range(QC):
            nc.tensor.transpose(
                rsT[:, c : c + 1],
                rs_sb[0:1, c * P : (c + 1) * P],
                ident[0:1, 0:1],
            )
        rcpT = stat.tile([P, QC], f32)
        nc.vector.reciprocal(rcpT[:], rsT[:])

        ot_sb = stat.tile([P, SQ], f32)
        o_ps = st_ps.tile([P, SQ], f32, tag="st")
        o_sb = stat.tile([P, SQ], f32)
        # staged: copies, then transposes, then mults — interleaving PE
        # writes with DVE reads of the same PSUM bank forces serialization
        for c in range(QC):
            sl = slice(c * P, (c + 1) * P)
            nc.scalar.copy(ot_sb[:, sl], ot[:, sl])
        for c in range(QC):
            sl = slice(c * P, (c + 1) * P)
            nc.tensor.transpose(o_ps[:, sl], ot_sb[:, sl], ident[:])
        for c in range(QC):
            sl = slice(c * P, (c + 1) * P)
            nc.vector.tensor_scalar_mul(o_sb[:, sl], o_ps[:, sl], rcpT[:, c : c + 1])
            nc.sync.dma_start(out[:, sl], o_sb[:, sl])

    nc.compile()
    return nc


_NC_CACHE = None


def kernel(inputs, mask, Wq, Wk, Wv):
    global _NC_CACHE, LAST_RESULT
    inputs = np.asarray(inputs)
    mask = np.asarray(mask)
    bf = ml_dtypes.bfloat16
    scale = np.float32(1.0 / math.sqrt(DK))

    def pack_w(w):  # [E, DK] -> [p, c*DK+d] = w[c*128+p, d]
        w = np.asarray(w).astype(bf)
        return np.ascontiguousarray(
            w.reshape(EC, P, DK).transpose(1, 0, 2).reshape(P, EC * DK)
        )

    wq_h = pack_w(np.asarray(Wq) * scale)
    wk_h = pack_w(Wk)
    wv_h = pack_w(Wv)

    if _NC_CACHE is None:
        _NC_CACHE = build()
    nc = _NC_CACHE

    in_maps = []
    for core in range(8):
        b, h = divmod(core, 2)
        q0 = h * SQ
        idx = np.r_[q0:S, 0:q0]  # rotate so this core's queries come first
        xb = inputs[b]  # [S, E] f32
        xt_core = np.ascontiguousarray(xb[idx].T).astype(bf)  # [E, S]
        mt_core = np.ascontiguousarray(
            mask[b, q0 : q0 + SQ, :][:, idx].T
        ).astype(bf)  # [S, SQ]
        in_maps.append(
            {"xt": xt_core, "wq": wq_h, "wk": wk_h, "wv": wv_h, "mt": mt_core}
        )

    res = run_bass_kernel_spmd(nc, in_maps, list(range(8)), trace=TRACE)
    LAST_RESULT = res
    outp = np.empty((B, S, DV), np.float32)
    for core in range(8):
        b, h = divmod(core, 2)
        q0 = h * SQ
        o = np.asarray(res.results[core]["out"])  # [P, SQ]
        # out[p, c*128 + d] = attention_out[q0 + c*128 + p, d]
        outp[b, q0 : q0 + SQ, :] = (
            o.reshape(P, QC, DV).transpose(1, 0, 2).reshape(SQ, DV)
        )
    return outp


# revision 34
# speedup vs baseline: 1.0124x; 1.0124x over previous
"""Distributed single-head attention for TRN2 (8 NeuronCores).

Reference computation (per batch b):
    q = x @ Wq; k = x @ Wk; v = x @ Wv          (x: [S, E])
    s = (q @ k.T) / sqrt(DK) - 1e15 * mask
    out = softmax(s, axis=-1) @ v               ([S, DV])

Sharding: 8 cores = 4 batches x 2 sequence halves. Each core computes
attention for 1024 queries of one batch; K/V are recomputed per core from
the full sequence (cheap vs. the attention matmuls, avoids collectives).

Per-core layout choices (host prepares these in kernel()):
  - xt  [E, S]  bf16: x_b^T with the sequence permuted so this core's
                query half occupies columns [0, 1024). K/V are computed
                over the permuted order, which is harmless because
                softmax-attention is permutation invariant over keys.
  - wq  [E, DK] bf16: Wq pre-scaled by 1/sqrt(DK).
  - mt  [S, SQ] bf16: mask[b, q0:q0+SQ, :] transposed to [key, query]
                with keys permuted identically to xt's columns.
  - out [P, SQ] f32: output queries grouped by 128: out[p, c*128+d] =
                attention_out[q0 + c*128 + p, d]; host re-interleaves.

On-core dataflow (all matmul contractions on the 128-partition dim):
  QT[d,q], KT[d,k], VT[d,k] projections -> DMA-transpose VT -> V[k,d]
  per key-tile t: ST[k128,q] = KT_t^T QT (PE) -> P = exp(ST) bf16 (ACT)
  -> zero masked entries (DVE copy_predicated) -> rowsum via ones-matmul
  and OT[dv,q] += V_t^T P (PE, PSUM accumulate). Epilogue transposes the
  rowsum into partition-major [128, 8] so the reciprocal runs on 8
  elements per lane (a [1, 1024] reciprocal costs 6.5us), then
  transposes OT chunks to [q, dv] and scales by 1/rowsum as a
  per-partition scalar.
"""

import math
from contextlib import ExitStack

import ml_dtypes
import numpy as np

import concourse.bass as bass
import concourse.bass_utils as _bass_utils
import concourse.tile as tile
from concourse import bacc, masks, mybir
from concourse.bass_utils import run_bass_kernel_spmd

# Note: tried forcing walrus --enable-ldw-opt=true to dedup the per-matmul
# LDWEIGHTS (~107ns each); walrus rejects it ("InstLdweights is not compatible
# with LDW optimization") because bacc emits standalone Ldweights when moving
# matmul waits. Keeping the default.
del _bass_utils

B, S, E, DK, DV = 4, 2048, 1024, 128, 128
SQ = S // 2  # queries per core
P = 128  # SBUF partitions
EC = E // P  # contraction chunks for projections
KTILES = S // P  # key tiles
QC = SQ // P  # query chunks (epilogue)

f32 = mybir.dt.float32
bf16 = mybir.dt.bfloat16

# test.py pokes these to get profiling info
TRACE = False
LAST_RESULT = None


def build():
    nc = bacc.Bacc()
    xt = nc.declare_dram_parameter("xt", [E, S], bf16, isOutput=False)
    # weights arrive host-packed as [p, c*DK+d] = W[c*128+p, d] so the load
    # is one fully-contiguous DMA (2KB/partition descriptors)
    wq = nc.declare_dram_parameter("wq", [P, EC * DK], bf16, isOutput=False)
    wk = nc.declare_dram_parameter("wk", [P, EC * DK], bf16, isOutput=False)
    wv = nc.declare_dram_parameter("wv", [P, EC * DV], bf16, isOutput=False)
    mt = nc.declare_dram_parameter("mt", [S, SQ], bf16, isOutput=False)
    out = nc.declare_dram_parameter("out", [P, SQ], f32, isOutput=True)

    with ExitStack() as ctx:
        tc = ctx.enter_context(tile.TileContext(nc))
        const_pool = ctx.enter_context(tc.tile_pool(name="const", bufs=1))
        in_pool = ctx.enter_context(tc.tile_pool(name="inputs", bufs=1))
        proj_sb = ctx.enter_context(tc.tile_pool(name="proj", bufs=1))
        p_pool = ctx.enter_context(tc.tile_pool(name="p", bufs=4))
        stat = ctx.enter_context(tc.tile_pool(name="stat", bufs=1))
        proj_ctx = ctx.enter_context(ExitStack())
        proj_ps = proj_ctx.enter_context(
            tc.tile_pool(name="proj_ps", bufs=8, space="PSUM")
        )

        ones_col = const_pool.tile([P, 1], bf16)
        nc.gpsimd.memset(ones_col[:], 1.0)
        zeros_p = const_pool.tile([P, SQ], bf16)
        nc.gpsimd.memset(zeros_p[:], 0.0)
        ident = const_pool.tile([P, P], f32)
        masks.make_identity(nc, ident[:])

        # --- weights + x chunks. One strided DMA per weight keeps the sync
        # sequencer's ~330ns/dma_start issue serialization off the critical
        # path; issue order wq, x0, wk, wv, x1.. matches first-use order.
        w_sb = {}
        for name, w in (("wq", wq), ("wk", wk), ("wv", wv)):
            wt = in_pool.tile([P, EC * DK], bf16, tag=name)
            w_sb[name] = wt

        x_sb = []
        for c in range(EC):
            xc = in_pool.tile([P, S], bf16, tag=f"x{c}")
            x_sb.append(xc)

        nc.sync.dma_start(w_sb["wq"][:], wq[:, :])
        nc.sync.dma_start(x_sb[0][:], xt[0:P, :])
        nc.sync.dma_start(w_sb["wk"][:], wk[:, :])
        nc.sync.dma_start(w_sb["wv"][:], wv[:, :])
        for c in range(1, EC):
            nc.sync.dma_start(x_sb[c][:], xt[c * P : (c + 1) * P, :])

        # --- projections: QT [d, q], KT [d, k], VT [d, k] (bf16 in SBUF) ---
        # Contraction chunk c is the outer loop so the first matmul only
        # needs x chunk 0, not all eight.
        qt_sb = proj_sb.tile([P, SQ], bf16)
        kt_sb = proj_sb.tile([P, S], bf16)
        vt_sb = proj_sb.tile([P, S], bf16)

        # QT and KT interleaved per x-chunk: ~1.9us of matmuls per chunk
        # keeps PE just behind the ~1.4us/chunk DMA arrival rate. VT follows
        # (x already resident by then).
        def alloc_ps(n):
            pss = []
            for j in range(n):
                ps = proj_ps.tile([P, 512], f32, tag="pps")
                pss.append(ps)
            return pss

        def proj_mm(pss, w_tile, c, nj):
            for j in range(nj):
                nc.tensor.matmul(
                    pss[j][:],
                    w_tile[:, c * DK : (c + 1) * DK],
                    x_sb[c][:, j * 512 : (j + 1) * 512],
                    start=(c == 0),
                    stop=(c == EC - 1),
                )

        def proj_copy(dst, pss):
            for j, ps in enumerate(pss):
                nc.vector.tensor_copy(dst[:, j * 512 : (j + 1) * 512], ps[:])

        qt_ps = alloc_ps(2)
        kt_ps = alloc_ps(4)
        for c in range(EC):
            proj_mm(qt_ps, w_sb["wq"], c, 2)
            proj_mm(kt_ps, w_sb["wk"], c, 4)
        proj_copy(qt_sb, qt_ps)
        proj_copy(kt_sb, kt_ps)
        vt_ps = alloc_ps(4)
        for c in range(EC):
            proj_mm(vt_ps, w_sb["wv"], c, 4)
        proj_copy(vt_sb, vt_ps)
        proj_ctx.close()  # free projection PSUM banks for the attention loop

        # --- V natural layout [k, dv] via DMA transpose on the sync stream
        # (sync has nothing left to issue afterward except output stores, so
        # blocking on vt_sb readiness is harmless) ---
        v_sb = proj_sb.tile([P, S], bf16)  # tile t at columns [t*DV, (t+1)*DV)
        for t in range(KTILES):
            nc.sync.dma_start_transpose(
                v_sb[:, t * DV : (t + 1) * DV], vt_sb[:, t * P : (t + 1) * P]
            )

        # --- mask loads on the GPSIMD SWDGE stream (third parallel issuer);
        # needed only from the attention loop onward ---
        m_sb = []
        for t in range(KTILES):
            mtile = in_pool.tile([P, SQ], bf16, tag=f"m{t}")
            m_sb.append(mtile)
            nc.gpsimd.dma_start(mtile[:], mt[t * P : (t + 1) * P, :])

        st_ps = ctx.enter_context(tc.tile_pool(name="st_ps", bufs=2, space="PSUM"))
        ot_ps = ctx.enter_context(tc.tile_pool(name="ot_ps", bufs=1, space="PSUM"))
        rs_pool = ctx.enter_context(tc.tile_pool(name="rs_ps", bufs=1, space="PSUM"))

        # --- attention over key tiles ---
        ot = ot_ps.tile([P, SQ], f32)  # OT [dv, q] accumulator
        rs = rs_pool.tile([1, SQ], f32)  # rowsum of masked exp(scores)
        for t in range(KTILES):
            st = st_ps.tile([P, SQ], f32, tag="st")  # [k128, q]
            for j in range(2):
                nc.tensor.matmul(
                    st[:, j * 512 : (j + 1) * 512],
                    kt_sb[:, t * P : (t + 1) * P],
                    qt_sb[:, j * 512 : (j + 1) * 512],
                    start=True,
                    stop=True,
                )
            p = p_pool.tile([P, SQ], bf16, tag="p")
            nc.scalar.activation(p[:], st[:], mybir.ActivationFunctionType.Exp)
            # zero the masked entries: exp(s - 1e15*m) == exp(s) * (1 - m)
            nc.vector.copy_predicated(
                p[:], m_sb[t][:].bitcast(mybir.dt.uint16), zeros_p[:]
            )
            for j in range(2):
                nc.tensor.matmul(
                    rs[:, j * 512 : (j + 1) * 512],
                    ones_col[:],
                    p[:, j * 512 : (j + 1) * 512],
                    start=(t == 0),
                    stop=(t == KTILES - 1),
                )
                nc.tensor.matmul(
                    ot[:, j * 512 : (j + 1) * 512],
                    v_sb[:, t * DV : (t + 1) * DV],
                    p[:, j * 512 : (j + 1) * 512],
                    start=(t == 0),
                    stop=(t == KTILES - 1),
                )

        # --- epilogue: normalize in [q, dv] layout ---
        # rowsum [1, SQ] -> SBUF -> PE-transpose to [128, QC] so reciprocal
        # runs on QC elements per lane instead of SQ on one lane.
        rs_sb = stat.tile([1, SQ], f32)
        nc.scalar.copy(rs_sb[:], rs[:])
        rsT = st_ps.tile([P, QC], f32, tag="st")
        for c in range(QC):
            nc.tensor.transpose(
                rsT[:, c : c + 1],
                rs_sb[0:1, c * P : (c + 1) * P],
                ident[0:1, 0:1],
            )
        rcpT = stat.tile([P, QC], f32)
        nc.vector.reciprocal(rcpT[:], rsT[:])

        ot_sb = stat.tile([P, SQ], f32)
        o_ps = st_ps.tile([P, SQ], f32, tag="st")
        o_sb = stat.tile([P, SQ], f32)
        # staged: copies, then transposes, then mults — interleaving PE
        # writes with DVE reads of the same PSUM bank forces serialization
        for c in range(QC):
            sl = slice(c * P, (c + 1) * P)
            nc.scalar.copy(ot_sb[:, sl], ot[:, sl])
        for c in range(QC):
            sl = slice(c * P, (c + 1) * P)
            nc.tensor.transpose(o_ps[:, sl], ot_sb[:, sl], ident[:])
        for c in range(QC):
            sl = slice(c * P, (c + 1) * P)
            nc.vector.tensor_scalar_mul(o_sb[:, sl], o_ps[:, sl], rcpT[:, c : c + 1])
            nc.sync.dma_start(out[:, sl], o_sb[:, sl])

    nc.compile()
    return nc


_NC_CACHE = None


def kernel(inputs, mask, Wq, Wk, Wv):
    global _NC_CACHE, LAST_RESULT
    inputs = np.asarray(inputs)
    mask = np.asarray(mask)
    bf = ml_dtypes.bfloat16
    scale = np.float32(1.0 / math.sqrt(DK))

    def pack_w(w):  # [E, DK] -> [p, c*DK+d] = w[c*128+p, d]
        w = np.asarray(w).astype(bf)
        return np.ascontiguousarray(
            w.reshape(EC, P, DK).transpose(1, 0, 2).reshape(P, EC * DK)
        )

    wq_h = pack_w(np.asarray(Wq) * scale)
    wk_h = pack_w(Wk)
    wv_h = pack_w(Wv)

    if _NC_CACHE is None:
        _NC_CACHE = build()
    nc = _NC_CACHE

    in_maps = []
    for core in range(8):
        b, h = divmod(core, 2)
        q0 = h * SQ
        idx = np.r_[q0:S, 0:q0]  # rotate so this core's queries come first
        xb = inputs[b]  # [S, E] f32
        xt_core = np.ascontiguousarray(xb[idx].T).astype(bf)  # [E, S]
        mt_core = np.ascontiguousarray(
            mask[b, q0 : q0 + SQ, :][:, idx].T
        ).astype(bf)  # [S, SQ]
        in_maps.append(
            {"xt": xt_core, "wq": wq_h, "wk": wk_h, "wv": wv_h, "mt": mt_core}
        )

    res = run_bass_kernel_spmd(nc, in_maps, list(range(8)), trace=TRACE)
    LAST_RESULT = res
    outp = np.empty((B, S, DV), np.float32)
    for core in range(8):
        b, h = divmod(core, 2)
        q0 = h * SQ
        o = np.asarray(res.results[core]["out"])  # [P, SQ]
        # out[p, c*128 + d] = attention_out[q0 + c*128 + p, d]
        outp[b, q0 : q0 + SQ, :] = (
            o.reshape(P, QC, DV).transpose(1, 0, 2).reshape(SQ, DV)
        )
    return outp
